# revision 20
# baseline (speedup 1.0000x reference)
"""Segment-mean (MeanAggregator) Trainium2 kernel.

Problem: atom_hiddens [2_000_000, 128] f32, segment_ids = repeat(arange(100_000), 20)
(uniform 20 atoms per molecule), output = per-molecule mean [100_000, 128] f32.

Strategy (8 NeuronCores, data-parallel over molecules):
  - Each core handles 12_500 molecules = 250_000 contiguous atom rows (128 MB).
  - Four-consecutive-molecules-per-partition layout on 128 partitions: in
    a full super-tile (512 mols, 5.24 MB), partition p holds molecules
    4p..4p+3 as 80 contiguous atom rows = ONE 40 KB contiguous HBM run
    (128 descriptor rows per DMA, each a full line-rate burst), and its
    four output rows are one contiguous 2 KB store run.  Small edge tiles
    use the plain one-mol-per-partition layout (10 KB runs).
  - The SBUF partition count is the first-order DMA knob: the HWDGE splits
    a DMA's descriptors across SDMA engines by partition, using the
    largest divisor of the partition count <= 16.  128 partitions -> all
    16 engines; a 125-partition variant of this kernel used only 5 engines
    and ran 2.6x slower.  Every tile here spans 128 partitions.
  - Input DMAs alternate between the two HWDGE rings (SP and ACT) so one
    ring's instruction boundary never idles the SDMA engines; output DMAs
    ride the SWDGE (gpsimd) ring, fully off the input path.
  - The 20-step reduction sum_r tile[p, g, r, :] alternates super-tiles
    between two engines, all fed from ONE shared 4-deep tile pool:
      * PE super-tiles: 20 accumulating fp32 identity matmuls into PSUM
        (partition-preserving), FD=512; ScalarE evicts with scale 1/20.
      * DVE super-tiles: tensor_reduce over the permuted AP [p, h, r]
        (axis=X) per group + fused 1/20 scalar multiply.
  - Work order tuned for pipeline edges: the 212-mol tail is loaded and
    reduced FIRST (128 mols on PE, 84 on DVE), 23 full super-tiles stream
    through the middle, and the last super-tile is four G=1 tiles ordered
    DVE/PE/DVE/PE so the drain after the final input byte is one small
    warm-PE tile (~4 us).
"""

import numpy as np

N_CORES = 8
TOTAL_ATOMS = 2_000_000
HIDDEN = 128
N_MOLS = 100_000
K = 20  # atoms per molecule
MOLS_PER_CORE = N_MOLS // N_CORES  # 12_500
ATOMS_PER_CORE = TOTAL_ATOMS // N_CORES  # 250_000

G = 4  # groups per super-tile
MOLS_PER_GROUP = 128
ATOMS_PER_GROUP = MOLS_PER_GROUP * K  # 2560
MOLS_PER_ST = G * MOLS_PER_GROUP  # 512
ATOMS_PER_ST = G * ATOMS_PER_GROUP  # 10240
N_ST = MOLS_PER_CORE // MOLS_PER_ST  # 24 full super-tiles
TAIL_MOLS = MOLS_PER_CORE - N_ST * MOLS_PER_ST  # 212
TAIL_A = 128  # tail mols on PE
TAIL_B = TAIL_MOLS - TAIL_A  # 84 tail mols on DVE

_CACHE = {}


def _build_program():
    import concourse.bacc as bacc
    import concourse.tile as tile
    from concourse import mybir

    nc = bacc.Bacc("TRN2", target_bir_lowering=False, debug=False)

    f32 = mybir.dt.float32

    x = nc.dram_tensor("x", [ATOMS_PER_CORE, HIDDEN], f32, kind="ExternalInput")
    ident = nc.dram_tensor("ident", [128, 128], f32, kind="ExternalInput")
    y = nc.dram_tensor("y", [MOLS_PER_CORE, HIDDEN], f32, kind="ExternalOutput")

    inv_k = 1.0 / K
    copy = mybir.ActivationFunctionType.Copy
    AX = mybir.AxisListType.X

    with tile.TileContext(nc) as tc:
        with (
            tc.tile_pool(name="constp", bufs=1) as constp,
            tc.tile_pool(name="inp", bufs=5) as inp,
            tc.tile_pool(name="outp", bufs=3) as outp,
            tc.tile_pool(name="psump", bufs=4, space="PSUM") as psump,
        ):
            ident_sb = constp.tile([128, 128], f32)
            nc.scalar.dma_start(out=ident_sb, in_=ident[:, :])

            ring = [nc.sync, nc.scalar]
            ring_i = 0

            def load_tile(a0, p, g):
                nonlocal ring_i
                in_t = inp.tile([128, g, K, HIDDEN], f32, tag="in")
                ring[ring_i % 2].dma_start(
                    out=in_t[:p],
                    in_=x[a0 : a0 + g * p * K, :].rearrange(
                        "(g p r) h -> p g r h", g=g, p=p, r=K
                    ),
                )
                ring_i += 1
                return in_t[:p]

            def store_tile(o_t, m0, p, g):
                nc.gpsimd.dma_start(
                    out=y[m0 : m0 + g * p, :].rearrange("(g p) h -> p g h", g=g, p=p),
                    in_=o_t,
                )

            def reduce_pe(in_t, m0, p, g):
                ps = psump.tile([128, 512], f32, tag="ps")
                fd = g * HIDDEN
                for r in range(K):
                    nc.tensor.matmul(
                        ps[:p, :fd],
                        lhsT=ident_sb[:p, :p],
                        rhs=in_t[:, :, r, :],
                        start=(r == 0),
                        stop=(r == K - 1),
                    )
                o_t = outp.tile([p, g, HIDDEN], f32, tag="out")
                nc.scalar.activation(o_t, ps[:p, :fd], copy, scale=inv_k)
                store_tile(o_t, m0, p, g)

            def reduce_dve(in_t, m0, p, g):
                o_t = outp.tile([p, g, HIDDEN], f32, tag="out")
                for j in range(g):
                    nc.vector.reduce_sum(
                        out=o_t[:, j, :],
                        in_=in_t[:, j, :, :].rearrange("p r h -> p h r"),
                        axis=AX,
                    )
                nc.vector.tensor_scalar_mul(o_t, o_t, inv_k)
                store_tile(o_t, m0, p, g)

            # ---- tail first: 212 mols, reduced while the pipe fills ----
            ta = N_ST * ATOMS_PER_ST
            tm = N_ST * MOLS_PER_ST
            in_a = load_tile(ta, 128, 1)
            in_b = load_tile(ta + ATOMS_PER_GROUP, TAIL_B, 1)
            reduce_pe(in_a, tm, 128, 1)
            reduce_dve(in_b, tm + TAIL_A, TAIL_B, 1)

            def load_tile_pairs(a0):
                """Super-tile with 2 consecutive mols per partition ->
                20 KB contiguous HBM runs (vs 10 KB), same SBUF layout."""
                nonlocal ring_i
                in_t = inp.tile([128, G, K, HIDDEN], f32, tag="in")
                ring[ring_i % 2].dma_start(
                    out=in_t,
                    in_=x[a0 : a0 + ATOMS_PER_ST, :].rearrange(
                        "(p q r) h -> p q r h", p=128, q=G, r=K
                    ),
                )
                ring_i += 1
                return in_t

            def store_tile_pairs(o_t, m0):
                nc.gpsimd.dma_start(
                    out=y[m0 : m0 + MOLS_PER_ST, :].rearrange(
                        "(p q) h -> p q h", p=128, q=G
                    ),
                    in_=o_t,
                )

            def reduce_pe_pairs(in_t, m0):
                ps = psump.tile([128, 512], f32, tag="ps")
                for r in range(K):
                    nc.tensor.matmul(
                        ps,
                        lhsT=ident_sb,
                        rhs=in_t[:, :, r, :],
                        start=(r == 0),
                        stop=(r == K - 1),
                    )
                o_t = outp.tile([128, G, HIDDEN], f32, tag="out")
                nc.scalar.activation(o_t, ps, copy, scale=inv_k)
                store_tile_pairs(o_t, m0)

            def reduce_dve_pairs(in_t, m0):
                o_t = outp.tile([128, G, HIDDEN], f32, tag="out")
                for j in range(G):
                    nc.vector.reduce_sum(
                        out=o_t[:, j, :],
                        in_=in_t[:, j, :, :].rearrange("p r h -> p h r"),
                        axis=AX,
                    )
                nc.vector.tensor_scalar_mul(o_t, o_t, inv_k)
                store_tile_pairs(o_t, m0)

            # ---- 23 full super-tiles ----
            for s in range(N_ST - 1):
                in_t = load_tile_pairs(s * ATOMS_PER_ST)
                m0 = s * MOLS_PER_ST
                if s % 2 == 0:
                    reduce_pe_pairs(in_t, m0)
                else:
                    reduce_dve_pairs(in_t, m0)

            # ---- last super-tile as four G=1 tiles: DVE/PE/DVE/PE so the
            # two DVE tiles never stack at the end and the final tile is a
            # small warm-PE reduction ----
            a0 = (N_ST - 1) * ATOMS_PER_ST
            m0 = (N_ST - 1) * MOLS_PER_ST
            for j in range(G):
                in_j = load_tile(a0 + j * ATOMS_PER_GROUP, 128, 1)
                mj = m0 + j * MOLS_PER_GROUP
                if j % 2 == 0:
                    reduce_dve(in_j, mj, 128, 1)
                else:
                    reduce_pe(in_j, mj, 128, 1)

    nc.finalize()
    return nc


def _get_program():
    if "nc" not in _CACHE:
        _CACHE["nc"] = _build_program()
    return _CACHE["nc"]


def _uniform_pattern(segment_ids: np.ndarray, n_mols: int) -> bool:
    if segment_ids.shape != (TOTAL_ATOMS,) or n_mols != N_MOLS:
        return False
    expect = np.repeat(np.arange(N_MOLS, dtype=segment_ids.dtype), K)
    return bool(np.array_equal(segment_ids, expect))


def _numpy_fallback(atom_hiddens, segment_ids, n_mols):
    """Correct-but-slow path for non-uniform segment layouts (sorted ids)."""
    ah = np.asarray(atom_hiddens, dtype=np.float32)
    sid = np.asarray(segment_ids).astype(np.int64)
    counts = np.bincount(sid, minlength=n_mols).astype(np.float32)
    boundaries = np.searchsorted(sid, np.arange(n_mols))
    sums = np.add.reduceat(ah, boundaries, axis=0)
    empty = counts == 0
    if empty.any():
        sums[empty] = 0.0
    return sums / np.maximum(counts, 1.0)[:, None]


def kernel(**inputs) -> np.ndarray:
    atom_hiddens = np.asarray(inputs["atom_hiddens"], dtype=np.float32)
    segment_ids = np.asarray(inputs["segment_ids"])
    n_mols = int(np.asarray(inputs["n_mols"]))

    if not _uniform_pattern(segment_ids, n_mols) or atom_hiddens.shape != (
        TOTAL_ATOMS,
        HIDDEN,
    ):
        return _numpy_fallback(atom_hiddens, segment_ids, n_mols)

    from concourse.bass_utils import run_bass_kernel_spmd

    nc = _get_program()
    ident = np.eye(128, dtype=np.float32)
    in_maps = [
        {
            "x": atom_hiddens[c * ATOMS_PER_CORE : (c + 1) * ATOMS_PER_CORE],
            "ident": ident,
        }
        for c in range(N_CORES)
    ]
    res = run_bass_kernel_spmd(nc, in_maps, core_ids=list(range(N_CORES)))
    return np.concatenate([r["y"] for r in res.results], axis=0)


if __name__ == "__main__":
    rng = np.random.default_rng(0)
    ah = rng.standard_normal((TOTAL_ATOMS, HIDDEN), dtype=np.float32)
    sid = np.repeat(np.arange(N_MOLS, dtype=np.int32), K)
    out = kernel(atom_hiddens=ah, segment_ids=sid, n_mols=N_MOLS)
    ref = ah.reshape(N_MOLS, K, HIDDEN).mean(axis=1)
    err = np.abs(out - ref).max() / max(np.abs(ref).max(), 1e-9)
    print("rel err:", err)


# revision 21
# speedup vs baseline: 1.0413x; 1.0413x over previous
"""Segment-mean (MeanAggregator) Trainium2 kernel.

Problem: atom_hiddens [2_000_000, 128] f32, segment_ids = repeat(arange(100_000), 20)
(uniform 20 atoms per molecule), output = per-molecule mean [100_000, 128] f32.

Strategy (8 NeuronCores, data-parallel over molecules):
  - Each core handles 12_500 molecules = 250_000 contiguous atom rows (128 MB).
  - Four-consecutive-molecules-per-partition layout on 128 partitions: in
    a full super-tile (512 mols, 5.24 MB), partition p holds molecules
    4p..4p+3 as 80 contiguous atom rows = ONE 40 KB contiguous HBM run
    (128 descriptor rows per DMA, each a full line-rate burst), and its
    four output rows are one contiguous 2 KB store run.  Small edge tiles
    use the plain one-mol-per-partition layout (10 KB runs).
  - The SBUF partition count is the first-order DMA knob: the HWDGE splits
    a DMA's descriptors across SDMA engines by partition, using the
    largest divisor of the partition count <= 16.  128 partitions -> all
    16 engines; a 125-partition variant of this kernel used only 5 engines
    and ran 2.6x slower.  Every tile here spans 128 partitions.
  - Input DMAs ride the SP HWDGE ring; output DMAs (and the identity
    preload) ride the ACT HWDGE ring.  SWDGE (gpsimd) is avoided entirely:
    its descriptor rings live on SBUF ports shared with SDMA engines 7/15,
    and routing stores through it measurably turned one SDMA engine into
    an ~80 us straggler that gated the whole core.
  - The 20-step reduction sum_r tile[p, g, r, :] alternates super-tiles
    between two engines, all fed from ONE shared 4-deep tile pool:
      * PE super-tiles: 20 accumulating fp32 identity matmuls into PSUM
        (partition-preserving), FD=512; ScalarE evicts with scale 1/20.
      * DVE super-tiles: tensor_reduce over the permuted AP [p, h, r]
        (axis=X) per group + fused 1/20 scalar multiply.
  - Work order tuned for pipeline edges: the 212-mol tail is loaded and
    reduced FIRST (128 mols on PE, 84 on DVE), 23 full super-tiles stream
    through the middle, and the last super-tile is four G=1 tiles ordered
    DVE/PE/DVE/PE so the drain after the final input byte is one small
    warm-PE tile (~4 us).
"""

import numpy as np

N_CORES = 8
TOTAL_ATOMS = 2_000_000
HIDDEN = 128
N_MOLS = 100_000
K = 20  # atoms per molecule
MOLS_PER_CORE = N_MOLS // N_CORES  # 12_500
ATOMS_PER_CORE = TOTAL_ATOMS // N_CORES  # 250_000

G = 4  # groups per super-tile
MOLS_PER_GROUP = 128
ATOMS_PER_GROUP = MOLS_PER_GROUP * K  # 2560
MOLS_PER_ST = G * MOLS_PER_GROUP  # 512
ATOMS_PER_ST = G * ATOMS_PER_GROUP  # 10240
N_ST = MOLS_PER_CORE // MOLS_PER_ST  # 24 full super-tiles
TAIL_MOLS = MOLS_PER_CORE - N_ST * MOLS_PER_ST  # 212
TAIL_A = 128  # tail mols on PE
TAIL_B = TAIL_MOLS - TAIL_A  # 84 tail mols on DVE

_CACHE = {}


def _build_program():
    import concourse.bacc as bacc
    import concourse.tile as tile
    from concourse import mybir

    nc = bacc.Bacc("TRN2", target_bir_lowering=False, debug=False)

    f32 = mybir.dt.float32

    x = nc.dram_tensor("x", [ATOMS_PER_CORE, HIDDEN], f32, kind="ExternalInput")
    ident = nc.dram_tensor("ident", [128, 128], f32, kind="ExternalInput")
    y = nc.dram_tensor("y", [MOLS_PER_CORE, HIDDEN], f32, kind="ExternalOutput")

    inv_k = 1.0 / K
    copy = mybir.ActivationFunctionType.Copy
    AX = mybir.AxisListType.X

    with tile.TileContext(nc) as tc:
        with (
            tc.tile_pool(name="constp", bufs=1) as constp,
            tc.tile_pool(name="inp", bufs=5) as inp,
            tc.tile_pool(name="outp", bufs=3) as outp,
            tc.tile_pool(name="psump", bufs=4, space="PSUM") as psump,
        ):
            ident_sb = constp.tile([128, 128], f32)
            nc.scalar.dma_start(out=ident_sb, in_=ident[:, :])

            ring = [nc.sync, nc.sync]
            ring_i = 0

            def load_tile(a0, p, g):
                nonlocal ring_i
                in_t = inp.tile([128, g, K, HIDDEN], f32, tag="in")
                ring[ring_i % 2].dma_start(
                    out=in_t[:p],
                    in_=x[a0 : a0 + g * p * K, :].rearrange(
                        "(g p r) h -> p g r h", g=g, p=p, r=K
                    ),
                )
                ring_i += 1
                return in_t[:p]

            def store_tile(o_t, m0, p, g):
                nc.scalar.dma_start(
                    out=y[m0 : m0 + g * p, :].rearrange("(g p) h -> p g h", g=g, p=p),
                    in_=o_t,
                )

            def reduce_pe(in_t, m0, p, g):
                ps = psump.tile([128, 512], f32, tag="ps")
                fd = g * HIDDEN
                for r in range(K):
                    nc.tensor.matmul(
                        ps[:p, :fd],
                        lhsT=ident_sb[:p, :p],
                        rhs=in_t[:, :, r, :],
                        start=(r == 0),
                        stop=(r == K - 1),
                    )
                o_t = outp.tile([p, g, HIDDEN], f32, tag="out")
                nc.scalar.activation(o_t, ps[:p, :fd], copy, scale=inv_k)
                store_tile(o_t, m0, p, g)

            def reduce_dve(in_t, m0, p, g):
                o_t = outp.tile([p, g, HIDDEN], f32, tag="out")
                for j in range(g):
                    nc.vector.reduce_sum(
                        out=o_t[:, j, :],
                        in_=in_t[:, j, :, :].rearrange("p r h -> p h r"),
                        axis=AX,
                    )
                nc.vector.tensor_scalar_mul(o_t, o_t, inv_k)
                store_tile(o_t, m0, p, g)

            # ---- tail first: 212 mols, reduced while the pipe fills ----
            ta = N_ST * ATOMS_PER_ST
            tm = N_ST * MOLS_PER_ST
            in_a = load_tile(ta, 128, 1)
            in_b = load_tile(ta + ATOMS_PER_GROUP, TAIL_B, 1)
            reduce_pe(in_a, tm, 128, 1)
            reduce_dve(in_b, tm + TAIL_A, TAIL_B, 1)

            def load_tile_pairs(a0):
                """Super-tile with 2 consecutive mols per partition ->
                20 KB contiguous HBM runs (vs 10 KB), same SBUF layout."""
                nonlocal ring_i
                in_t = inp.tile([128, G, K, HIDDEN], f32, tag="in")
                ring[ring_i % 2].dma_start(
                    out=in_t,
                    in_=x[a0 : a0 + ATOMS_PER_ST, :].rearrange(
                        "(p q r) h -> p q r h", p=128, q=G, r=K
                    ),
                )
                ring_i += 1
                return in_t

            def store_tile_pairs(o_t, m0):
                nc.scalar.dma_start(
                    out=y[m0 : m0 + MOLS_PER_ST, :].rearrange(
                        "(p q) h -> p q h", p=128, q=G
                    ),
                    in_=o_t,
                )

            def reduce_pe_pairs(in_t, m0):
                ps = psump.tile([128, 512], f32, tag="ps")
                for r in range(K):
                    nc.tensor.matmul(
                        ps,
                        lhsT=ident_sb,
                        rhs=in_t[:, :, r, :],
                        start=(r == 0),
                        stop=(r == K - 1),
                    )
                o_t = outp.tile([128, G, HIDDEN], f32, tag="out")
                nc.scalar.activation(o_t, ps, copy, scale=inv_k)
                store_tile_pairs(o_t, m0)

            def reduce_dve_pairs(in_t, m0):
                o_t = outp.tile([128, G, HIDDEN], f32, tag="out")
                for j in range(G):
                    nc.vector.reduce_sum(
                        out=o_t[:, j, :],
                        in_=in_t[:, j, :, :].rearrange("p r h -> p h r"),
                        axis=AX,
                    )
                nc.vector.tensor_scalar_mul(o_t, o_t, inv_k)
                store_tile_pairs(o_t, m0)

            # ---- 23 full super-tiles ----
            for s in range(N_ST - 1):
                in_t = load_tile_pairs(s * ATOMS_PER_ST)
                m0 = s * MOLS_PER_ST
                if s % 2 == 0:
                    reduce_pe_pairs(in_t, m0)
                else:
                    reduce_dve_pairs(in_t, m0)

            # ---- last super-tile as four G=1 tiles: DVE/PE/DVE/PE so the
            # two DVE tiles never stack at the end and the final tile is a
            # small warm-PE reduction ----
            a0 = (N_ST - 1) * ATOMS_PER_ST
            m0 = (N_ST - 1) * MOLS_PER_ST
            for j in range(G):
                in_j = load_tile(a0 + j * ATOMS_PER_GROUP, 128, 1)
                mj = m0 + j * MOLS_PER_GROUP
                if j % 2 == 0:
                    reduce_dve(in_j, mj, 128, 1)
                else:
                    reduce_pe(in_j, mj, 128, 1)

    nc.finalize()
    return nc


def _get_program():
    if "nc" not in _CACHE:
        _CACHE["nc"] = _build_program()
    return _CACHE["nc"]


def _uniform_pattern(segment_ids: np.ndarray, n_mols: int) -> bool:
    if segment_ids.shape != (TOTAL_ATOMS,) or n_mols != N_MOLS:
        return False
    expect = np.repeat(np.arange(N_MOLS, dtype=segment_ids.dtype), K)
    return bool(np.array_equal(segment_ids, expect))


def _numpy_fallback(atom_hiddens, segment_ids, n_mols):
    """Correct-but-slow path for non-uniform segment layouts (sorted ids)."""
    ah = np.asarray(atom_hiddens, dtype=np.float32)
    sid = np.asarray(segment_ids).astype(np.int64)
    counts = np.bincount(sid, minlength=n_mols).astype(np.float32)
    boundaries = np.searchsorted(sid, np.arange(n_mols))
    sums = np.add.reduceat(ah, boundaries, axis=0)
    empty = counts == 0
    if empty.any():
        sums[empty] = 0.0
    return sums / np.maximum(counts, 1.0)[:, None]


def kernel(**inputs) -> np.ndarray:
    atom_hiddens = np.asarray(inputs["atom_hiddens"], dtype=np.float32)
    segment_ids = np.asarray(inputs["segment_ids"])
    n_mols = int(np.asarray(inputs["n_mols"]))

    if not _uniform_pattern(segment_ids, n_mols) or atom_hiddens.shape != (
        TOTAL_ATOMS,
        HIDDEN,
    ):
        return _numpy_fallback(atom_hiddens, segment_ids, n_mols)

    from concourse.bass_utils import run_bass_kernel_spmd

    nc = _get_program()
    ident = np.eye(128, dtype=np.float32)
    in_maps = [
        {
            "x": atom_hiddens[c * ATOMS_PER_CORE : (c + 1) * ATOMS_PER_CORE],
            "ident": ident,
        }
        for c in range(N_CORES)
    ]
    res = run_bass_kernel_spmd(nc, in_maps, core_ids=list(range(N_CORES)))
    return np.concatenate([r["y"] for r in res.results], axis=0)


if __name__ == "__main__":
    rng = np.random.default_rng(0)
    ah = rng.standard_normal((TOTAL_ATOMS, HIDDEN), dtype=np.float32)
    sid = np.repeat(np.arange(N_MOLS, dtype=np.int32), K)
    out = kernel(atom_hiddens=ah, segment_ids=sid, n_mols=N_MOLS)
    ref = ah.reshape(N_MOLS, K, HIDDEN).mean(axis=1)
    err = np.abs(out - ref).max() / max(np.abs(ref).max(), 1e-9)
    print("rel err:", err)
